# revision 7
# baseline (speedup 1.0000x reference)
"""HardClusterAssigner Trainium2 kernel.

Reference computation:
    x_emb = mean_b(einsum('bsv,hs->bvh', x, W) + b)   # [V, H]
    assignments = one_hot(argmin(-l2norm(x_emb) @ l2norm(centroids).T))

Key transformations used here:
  1. mean over B commutes with the (linear) contraction over S:
         mean_b(x @ W.T) = (mean_b x) @ W.T
     so the 34-GFLOP batched matmul collapses to a memory-bound reduction
     of x over B followed by one [V,S]x[S,H] matmul.
  2. l2norm of the embedding is a positive per-row scale -> it cannot change
     the row-wise argmin, so it is skipped. Only centroids need normalizing.
  3. The 1/B mean scale and the bias are folded in exactly:
         B * (mean_b(xW.T) + bias) = (sum_b x) @ W.T + B*bias
     and the overall positive factor B is again argmin-invariant.

Sharding: V (last dim of x) is split across the 8 cores; every stage after
the split is core-local (no collectives). Each core computes its 64 rows of
the one-hot output.
"""

import sys

for _p in ("/opt/trn_rl_repo",):
    if _p not in sys.path:
        sys.path.append(_p)

from contextlib import ExitStack

import numpy as np

import concourse.bacc as bacc
import concourse.bass as bass
import concourse.mybir as mybir
from concourse import tile
from concourse.bass_utils import run_bass_kernel_spmd
from concourse.masks import make_identity

B, S, V, H, C = 64, 1024, 512, 512, 64
NCORES = 8
VL = V // NCORES  # 64 V-columns per core
P = 128
ST = S // P  # 8 s-chunks
F32 = mybir.dt.float32

_NC_CACHE = None


def build_bass() -> bass.Bass:
    nc = bacc.Bacc("TRN2", target_bir_lowering=False)

    xs = nc.declare_dram_parameter("xs", [S, B, VL], F32, isOutput=False)
    wt = nc.declare_dram_parameter("wt", [S, H], F32, isOutput=False)
    bb = nc.declare_dram_parameter("bb", [1, H], F32, isOutput=False)
    cent = nc.declare_dram_parameter("cent", [C, H], F32, isOutput=False)
    out = nc.declare_dram_parameter("out", [VL, C], F32, isOutput=True)

    with tile.TileContext(nc) as tc, ExitStack() as ctx:
        consts = ctx.enter_context(tc.tile_pool(name="consts", bufs=1))
        xpool = ctx.enter_context(tc.tile_pool(name="x", bufs=3))
        xmpool = ctx.enter_context(tc.tile_pool(name="xm", bufs=1))
        spool = ctx.enter_context(tc.tile_pool(name="small", bufs=1))
        psum = ctx.enter_context(tc.tile_pool(name="psum", bufs=1, space="PSUM"))
        tpsum = ctx.enter_context(tc.tile_pool(name="tpsum", bufs=2, space="PSUM"))

        # --- constants / small inputs -------------------------------------
        wtr = consts.tile([P, ST, H], F32)  # W.T tiled: [:, t, :] is s-chunk t
        nc.sync.dma_start(out=wtr[:], in_=wt.rearrange("(t p) h -> p t h", p=P))

        brow = consts.tile([1, H], F32)
        nc.sync.dma_start(out=brow[:], in_=bb[:])
        ones_row = consts.tile([1, VL], F32)
        nc.vector.memset(ones_row[:], 1.0)

        ident = consts.tile([P, P], F32)
        make_identity(nc, ident[:])

        centt = spool.tile([C, H], F32)
        nc.sync.dma_start(out=centt[:], in_=cent[:])

        # --- normalize centroids (rows) -----------------------------------
        csq = spool.tile([C, H], F32)
        ssq = spool.tile([C, 1], F32)
        nc.scalar.activation(
            csq[:], centt[:], mybir.ActivationFunctionType.Square, accum_out=ssq[:]
        )
        cnorm = spool.tile([C, 1], F32)
        nc.scalar.sqrt(cnorm[:], ssq[:])
        cinv = spool.tile([C, 1], F32)
        nc.vector.reciprocal(cinv[:], cnorm[:])
        centn = spool.tile([C, H], F32)
        nc.vector.tensor_scalar_mul(centn[:], centt[:], cinv[:])

        # centroids transposed to [H, C] chunks for the similarity matmul
        cenT = spool.tile([P, 4 * C], F32)
        for k in range(4):
            cp = tpsum.tile([P, C], F32, tag="tp")
            nc.tensor.transpose(cp[:], centn[:, k * P : (k + 1) * P], ident[:C, :C])
            nc.scalar.copy(cenT[:, k * C : (k + 1) * C], cp[:])

        # --- x: DMA + reduce over B, then accumulate matmuls --------------
        # xs[s, b, v]; tile t holds s in [t*128, (t+1)*128)
        xs_r = xs.rearrange("(t p) b v -> t p (b v)", p=P)
        emb_ps = psum.tile([VL, H], F32, tag="emb")
        # bias first (opens the accumulation group): ones.T @ (B*b)
        nc.tensor.matmul(emb_ps[:], ones_row[:], brow[:], start=True, stop=False)
        for t in range(ST):
            xt = xpool.tile([P, B * VL], F32, tag="xt")
            nc.sync.dma_start(out=xt[:], in_=xs_r[t])
            xm = xmpool.tile([P, VL], F32, tag=f"xm{t}")
            # sum over b: present free dims as (v, b) and reduce innermost
            nc.vector.tensor_reduce(
                xm[:],
                xt[:].rearrange("p (b v) -> p v b", b=B),
                axis=mybir.AxisListType.X,
                op=mybir.AluOpType.add,
            )
            nc.tensor.matmul(
                emb_ps[:],
                xm[:],
                wtr[:, t, :],
                start=False,
                stop=(t == ST - 1),
            )

        emb_sb = spool.tile([VL, H], F32)
        nc.scalar.copy(emb_sb[:], emb_ps[:])

        # --- similarity = emb @ centn.T via PE (contract H on partitions) -
        embT = spool.tile([P, 4 * VL], F32)
        for k in range(4):
            ep = tpsum.tile([P, VL], F32, tag="tp")
            nc.tensor.transpose(ep[:], emb_sb[:, k * P : (k + 1) * P], ident[:VL, :VL])
            nc.scalar.copy(embT[:, k * VL : (k + 1) * VL], ep[:])

        sim_ps = psum.tile([VL, C], F32, tag="sim")
        for k in range(4):
            nc.tensor.matmul(
                sim_ps[:],
                embT[:, k * VL : (k + 1) * VL],
                cenT[:, k * C : (k + 1) * C],
                start=(k == 0),
                stop=(k == 3),
            )

        # --- one-hot of row argmax ----------------------------------------
        mx = spool.tile([VL, 1], F32)
        nc.vector.tensor_reduce(
            mx[:], sim_ps[:], axis=mybir.AxisListType.X, op=mybir.AluOpType.max
        )
        oh = spool.tile([VL, C], F32)
        nc.vector.tensor_scalar(
            oh[:], sim_ps[:], mx[:], None, op0=mybir.AluOpType.is_equal
        )
        nc.sync.dma_start(out=out[:], in_=oh[:])

    nc.compile()
    return nc


def _get_nc() -> bass.Bass:
    global _NC_CACHE
    if _NC_CACHE is None:
        _NC_CACHE = build_bass()
    return _NC_CACHE


def make_in_maps(x, W, b, centroids):
    x = np.asarray(x, dtype=np.float32)
    W = np.asarray(W, dtype=np.float32)
    b = np.asarray(b, dtype=np.float32)
    centroids = np.asarray(centroids, dtype=np.float32)

    wt_host = np.ascontiguousarray(W.T)  # [S, H]
    brow = (np.float32(B) * b).reshape(1, H).astype(np.float32)
    cent_host = np.ascontiguousarray(centroids)

    in_maps = []
    for i in range(NCORES):
        xs_i = x[:, :, i * VL : (i + 1) * VL].transpose(1, 0, 2)  # [S, B, VL] view
        in_maps.append({"xs": xs_i, "wt": wt_host, "bb": brow, "cent": cent_host})
    return in_maps


def run(inputs: dict, trace: bool = False):
    """Run on the 8 NeuronCores; returns (full_output, BassKernelResults)."""
    nc = _get_nc()
    in_maps = make_in_maps(**inputs)
    res = run_bass_kernel_spmd(nc, in_maps, list(range(NCORES)), trace=trace)
    full = np.concatenate([r["out"] for r in res.results], axis=0)
    return full, res


def kernel(x, W, b, centroids) -> np.ndarray:
    full, _ = run({"x": x, "W": W, "b": b, "centroids": centroids})
    return full


# revision 12
# speedup vs baseline: 1.0536x; 1.0536x over previous
"""HardClusterAssigner Trainium2 kernel.

Reference computation:
    x_emb = mean_b(einsum('bsv,hs->bvh', x, W) + b)   # [V, H]
    assignments = one_hot(argmin(-l2norm(x_emb) @ l2norm(centroids).T))

Key transformations used here:
  1. mean over B commutes with the (linear) contraction over S:
         mean_b(x @ W.T) = (mean_b x) @ W.T
     so the 34-GFLOP batched matmul collapses to a memory-bound reduction
     of x over B followed by one [V,S]x[S,H] matmul.
  2. l2norm of the embedding is a positive per-row scale -> it cannot change
     the row-wise argmin, so it is skipped. Only centroids need normalizing.
  3. The 1/B mean scale and the bias are folded in exactly:
         B * (mean_b(xW.T) + bias) = (sum_b x) @ W.T + B*bias
     and the overall positive factor B is again argmin-invariant.

Sharding: V (last dim of x) is split across the 8 cores; every stage after
the split is core-local (no collectives). Each core computes its 64 rows of
the one-hot output.
"""

import sys

for _p in ("/opt/trn_rl_repo",):
    if _p not in sys.path:
        sys.path.append(_p)

from contextlib import ExitStack

import numpy as np

import concourse.bacc as bacc
import concourse.bass as bass
import concourse.mybir as mybir
from concourse import tile
from concourse.bass_utils import run_bass_kernel_spmd
from concourse.masks import make_identity

B, S, V, H, C = 64, 1024, 512, 512, 64
NCORES = 8
VL = V // NCORES  # 64 V-columns per core
P = 128
ST = S // P  # 8 s-chunks
F32 = mybir.dt.float32

_NC_CACHE = None


def build_bass() -> bass.Bass:
    nc = bacc.Bacc("TRN2", target_bir_lowering=False)

    xs = nc.declare_dram_parameter("xs", [S, VL, B], F32, isOutput=False)
    wt = nc.declare_dram_parameter("wt", [S, H], F32, isOutput=False)
    bb = nc.declare_dram_parameter("bb", [1, H], F32, isOutput=False)
    cent = nc.declare_dram_parameter("cent", [C, H], F32, isOutput=False)
    out = nc.declare_dram_parameter("out", [VL, C], F32, isOutput=True)

    with tile.TileContext(nc) as tc, ExitStack() as ctx:
        consts = ctx.enter_context(tc.tile_pool(name="consts", bufs=1))
        xpool = ctx.enter_context(tc.tile_pool(name="x", bufs=3))
        xmpool = ctx.enter_context(tc.tile_pool(name="xm", bufs=1))
        spool = ctx.enter_context(tc.tile_pool(name="small", bufs=1))
        psum = ctx.enter_context(tc.tile_pool(name="psum", bufs=1, space="PSUM"))
        tpsum = ctx.enter_context(tc.tile_pool(name="tpsum", bufs=2, space="PSUM"))

        # --- constants / small inputs -------------------------------------
        # const DMAs ride the ACT HWDGE ring so x tiles own the SP ring
        wtr = consts.tile([P, ST, H], F32)  # W.T tiled: [:, t, :] is s-chunk t
        nc.scalar.dma_start(out=wtr[:], in_=wt.rearrange("(t p) h -> p t h", p=P))

        brow = consts.tile([1, H], F32)
        nc.scalar.dma_start(out=brow[:], in_=bb[:])
        ones_row = consts.tile([1, VL], F32)
        nc.vector.memset(ones_row[:], 1.0)

        ident = consts.tile([P, P], F32)
        make_identity(nc, ident[:])

        centt = spool.tile([C, H], F32)
        nc.scalar.dma_start(out=centt[:], in_=cent[:])

        # --- normalize centroids (rows) -----------------------------------
        csq = spool.tile([C, H], F32)
        ssq = spool.tile([C, 1], F32)
        nc.scalar.activation(
            csq[:], centt[:], mybir.ActivationFunctionType.Square, accum_out=ssq[:]
        )
        cnorm = spool.tile([C, 1], F32)
        nc.scalar.sqrt(cnorm[:], ssq[:])
        cinv = spool.tile([C, 1], F32)
        nc.vector.reciprocal(cinv[:], cnorm[:])
        centn = spool.tile([C, H], F32)
        nc.vector.tensor_scalar_mul(centn[:], centt[:], cinv[:])

        # centroids transposed to [H, C] chunks for the similarity matmul
        cenT = spool.tile([P, 4 * C], F32)
        for k in range(4):
            cp = tpsum.tile([P, C], F32, tag="tp")
            nc.tensor.transpose(cp[:], centn[:, k * P : (k + 1) * P], ident[:C, :C])
            nc.scalar.copy(cenT[:, k * C : (k + 1) * C], cp[:])

        # --- x: DMA + reduce over B, then accumulate matmuls --------------
        # xs[s, v, b]; tile t holds s in [t*128, (t+1)*128); b innermost so
        # the reduce streams unit-stride. Two v-halves per s-chunk (1MiB DMAs)
        # for finer DMA/DVE pipelining.
        HV = VL // 2  # 32
        xs_r = xs.rearrange("(t p) v b -> t p (v b)", p=P)
        emb_ps = psum.tile([VL, H], F32, tag="emb")
        # bias first (opens the accumulation group): ones.T @ (B*b)
        nc.tensor.matmul(emb_ps[:], ones_row[:], brow[:], start=True, stop=False)
        for t in range(ST):
            xm = xmpool.tile([P, VL], F32, tag=f"xm{t}")
            for h in range(2):
                xt = xpool.tile([P, HV * B], F32, tag="xt")
                nc.sync.dma_start(out=xt[:], in_=xs_r[t][:, h * HV * B : (h + 1) * HV * B])
                nc.vector.tensor_reduce(
                    xm[:, h * HV : (h + 1) * HV],
                    xt[:].rearrange("p (v b) -> p v b", b=B),
                    axis=mybir.AxisListType.X,
                    op=mybir.AluOpType.add,
                )
            nc.tensor.matmul(
                emb_ps[:],
                xm[:],
                wtr[:, t, :],
                start=False,
                stop=(t == ST - 1),
            )

        emb_sb = spool.tile([VL, H], F32)
        nc.scalar.copy(emb_sb[:], emb_ps[:])

        # --- similarity = emb @ centn.T via PE (contract H on partitions) -
        embT = spool.tile([P, 4 * VL], F32)
        for k in range(4):
            ep = tpsum.tile([P, VL], F32, tag="tp")
            nc.tensor.transpose(ep[:], emb_sb[:, k * P : (k + 1) * P], ident[:VL, :VL])
            nc.scalar.copy(embT[:, k * VL : (k + 1) * VL], ep[:])

        sim_ps = psum.tile([VL, C], F32, tag="sim")
        for k in range(4):
            nc.tensor.matmul(
                sim_ps[:],
                embT[:, k * VL : (k + 1) * VL],
                cenT[:, k * C : (k + 1) * C],
                start=(k == 0),
                stop=(k == 3),
            )

        # --- one-hot of row argmax ----------------------------------------
        mx = spool.tile([VL, 1], F32)
        nc.vector.tensor_reduce(
            mx[:], sim_ps[:], axis=mybir.AxisListType.X, op=mybir.AluOpType.max
        )
        oh = spool.tile([VL, C], F32)
        nc.vector.tensor_scalar(
            oh[:], sim_ps[:], mx[:], None, op0=mybir.AluOpType.is_equal
        )
        nc.sync.dma_start(out=out[:], in_=oh[:])

    nc.compile()
    return nc


def _get_nc() -> bass.Bass:
    global _NC_CACHE
    if _NC_CACHE is None:
        _NC_CACHE = build_bass()
    return _NC_CACHE


def make_in_maps(x, W, b, centroids):
    x = np.asarray(x, dtype=np.float32)
    W = np.asarray(W, dtype=np.float32)
    b = np.asarray(b, dtype=np.float32)
    centroids = np.asarray(centroids, dtype=np.float32)

    wt_host = np.ascontiguousarray(W.T)  # [S, H]
    brow = (np.float32(B) * b).reshape(1, H).astype(np.float32)
    cent_host = np.ascontiguousarray(centroids)

    # Two-step host transpose [B,S,V] -> [S,V,B]: one pass to [S,B,V]
    # (contiguous 2KB runs, fast), then per-s [B,VL] -> [VL,B] blocks that
    # stay cache-resident. Direct one-shot transpose would thrash DRAM.
    xsb = np.ascontiguousarray(x.transpose(1, 0, 2))  # [S, B, V]
    in_maps = []
    for i in range(NCORES):
        xs_i = np.ascontiguousarray(
            xsb[:, :, i * VL : (i + 1) * VL].transpose(0, 2, 1)
        )  # [S, VL, B]
        in_maps.append({"xs": xs_i, "wt": wt_host, "bb": brow, "cent": cent_host})
    return in_maps


def run(inputs: dict, trace: bool = False):
    """Run on the 8 NeuronCores; returns (full_output, BassKernelResults)."""
    nc = _get_nc()
    in_maps = make_in_maps(**inputs)
    res = run_bass_kernel_spmd(nc, in_maps, list(range(NCORES)), trace=trace)
    full = np.concatenate([r["out"] for r in res.results], axis=0)
    return full, res


def kernel(x, W, b, centroids) -> np.ndarray:
    full, _ = run({"x": x, "W": W, "b": b, "centroids": centroids})
    return full


# revision 13
# speedup vs baseline: 1.1321x; 1.0745x over previous
"""HardClusterAssigner Trainium2 kernel.

Reference computation:
    x_emb = mean_b(einsum('bsv,hs->bvh', x, W) + b)   # [V, H]
    assignments = one_hot(argmin(-l2norm(x_emb) @ l2norm(centroids).T))

Key transformations used here:
  1. mean over B commutes with the (linear) contraction over S:
         mean_b(x @ W.T) = (mean_b x) @ W.T
     so the 34-GFLOP batched matmul collapses to a memory-bound reduction
     of x over B followed by one [V,S]x[S,H] matmul.
  2. l2norm of the embedding is a positive per-row scale -> it cannot change
     the row-wise argmin, so it is skipped. Only centroids need normalizing.
  3. The 1/B mean scale and the bias are folded in exactly:
         B * (mean_b(xW.T) + bias) = (sum_b x) @ W.T + B*bias
     and the overall positive factor B is again argmin-invariant.

Sharding: V (last dim of x) is split across the 8 cores; every stage after
the split is core-local (no collectives). Each core computes its 64 rows of
the one-hot output.
"""

import sys

for _p in ("/opt/trn_rl_repo",):
    if _p not in sys.path:
        sys.path.append(_p)

from contextlib import ExitStack

import numpy as np

import concourse.bacc as bacc
import concourse.bass as bass
import concourse.mybir as mybir
from concourse import tile
from concourse.bass_utils import run_bass_kernel_spmd
from concourse.masks import make_identity

B, S, V, H, C = 64, 1024, 512, 512, 64
NCORES = 8
VL = V // NCORES  # 64 V-columns per core
P = 128
ST = S // P  # 8 s-chunks
F32 = mybir.dt.float32

_NC_CACHE = None


def build_bass() -> bass.Bass:
    nc = bacc.Bacc("TRN2", target_bir_lowering=False)

    xs = nc.declare_dram_parameter("xs", [S, VL, B], F32, isOutput=False)
    wt = nc.declare_dram_parameter("wt", [S, H], F32, isOutput=False)
    bb = nc.declare_dram_parameter("bb", [1, H], F32, isOutput=False)
    cent = nc.declare_dram_parameter("cent", [C, H], F32, isOutput=False)
    out = nc.declare_dram_parameter("out", [VL, C], F32, isOutput=True)

    with tile.TileContext(nc) as tc, ExitStack() as ctx:
        consts = ctx.enter_context(tc.tile_pool(name="consts", bufs=1))
        xpool = ctx.enter_context(tc.tile_pool(name="x", bufs=8))
        xmpool = ctx.enter_context(tc.tile_pool(name="xm", bufs=1))
        spool = ctx.enter_context(tc.tile_pool(name="small", bufs=1))
        psum = ctx.enter_context(tc.tile_pool(name="psum", bufs=1, space="PSUM"))
        tpsum = ctx.enter_context(tc.tile_pool(name="tpsum", bufs=2, space="PSUM"))

        # --- constants / small inputs -------------------------------------
        # const DMAs ride the ACT HWDGE ring so x tiles own the SP ring
        wtr = consts.tile([P, ST, H], F32)  # W.T tiled: [:, t, :] is s-chunk t
        nc.scalar.dma_start(out=wtr[:], in_=wt.rearrange("(t p) h -> p t h", p=P))

        brow = consts.tile([1, H], F32)
        nc.scalar.dma_start(out=brow[:], in_=bb[:])
        ones_row = consts.tile([1, VL], F32)
        nc.vector.memset(ones_row[:], 1.0)

        ident = consts.tile([P, P], F32)
        make_identity(nc, ident[:])

        centt = spool.tile([C, H], F32)
        nc.scalar.dma_start(out=centt[:], in_=cent[:])

        # --- normalize centroids (rows) -----------------------------------
        csq = spool.tile([C, H], F32)
        ssq = spool.tile([C, 1], F32)
        nc.scalar.activation(
            csq[:], centt[:], mybir.ActivationFunctionType.Square, accum_out=ssq[:]
        )
        cnorm = spool.tile([C, 1], F32)
        nc.scalar.sqrt(cnorm[:], ssq[:])
        cinv = spool.tile([C, 1], F32)
        nc.vector.reciprocal(cinv[:], cnorm[:])
        centn = spool.tile([C, H], F32)
        nc.vector.tensor_scalar_mul(centn[:], centt[:], cinv[:])

        # centroids transposed to [H, C] chunks for the similarity matmul
        cenT = spool.tile([P, 4 * C], F32)
        for k in range(4):
            cp = tpsum.tile([P, C], F32, tag="tp")
            nc.tensor.transpose(cp[:], centn[:, k * P : (k + 1) * P], ident[:C, :C])
            nc.scalar.copy(cenT[:, k * C : (k + 1) * C], cp[:])

        # --- x: DMA + reduce over B, then accumulate matmuls --------------
        # xs[s, v, b]; tile t holds s in [t*128, (t+1)*128); b innermost so
        # the reduce streams unit-stride. Two v-halves per s-chunk (1MiB DMAs)
        # for finer DMA/DVE pipelining.
        HV = VL // 2  # 32
        xs_r = xs.rearrange("(t p) v b -> t p (v b)", p=P)
        emb_ps = psum.tile([VL, H], F32, tag="emb")
        # bias first (opens the accumulation group): ones.T @ (B*b)
        nc.tensor.matmul(emb_ps[:], ones_row[:], brow[:], start=True, stop=False)
        for t in range(ST):
            xm = xmpool.tile([P, VL], F32, tag=f"xm{t}")
            for h in range(2):
                xt = xpool.tile([P, HV * B], F32, tag="xt")
                nc.sync.dma_start(out=xt[:], in_=xs_r[t][:, h * HV * B : (h + 1) * HV * B])
                nc.vector.tensor_reduce(
                    xm[:, h * HV : (h + 1) * HV],
                    xt[:].rearrange("p (v b) -> p v b", b=B),
                    axis=mybir.AxisListType.X,
                    op=mybir.AluOpType.add,
                )
            nc.tensor.matmul(
                emb_ps[:],
                xm[:],
                wtr[:, t, :],
                start=False,
                stop=(t == ST - 1),
            )

        emb_sb = spool.tile([VL, H], F32)
        nc.scalar.copy(emb_sb[:], emb_ps[:])

        # --- similarity = emb @ centn.T via PE (contract H on partitions) -
        embT = spool.tile([P, 4 * VL], F32)
        for k in range(4):
            ep = tpsum.tile([P, VL], F32, tag="tp")
            nc.tensor.transpose(ep[:], emb_sb[:, k * P : (k + 1) * P], ident[:VL, :VL])
            nc.scalar.copy(embT[:, k * VL : (k + 1) * VL], ep[:])

        sim_ps = psum.tile([VL, C], F32, tag="sim")
        for k in range(4):
            nc.tensor.matmul(
                sim_ps[:],
                embT[:, k * VL : (k + 1) * VL],
                cenT[:, k * C : (k + 1) * C],
                start=(k == 0),
                stop=(k == 3),
            )

        # --- one-hot of row argmax ----------------------------------------
        mx = spool.tile([VL, 1], F32)
        nc.vector.tensor_reduce(
            mx[:], sim_ps[:], axis=mybir.AxisListType.X, op=mybir.AluOpType.max
        )
        oh = spool.tile([VL, C], F32)
        nc.vector.tensor_scalar(
            oh[:], sim_ps[:], mx[:], None, op0=mybir.AluOpType.is_equal
        )
        nc.sync.dma_start(out=out[:], in_=oh[:])

    nc.compile()
    return nc


def _get_nc() -> bass.Bass:
    global _NC_CACHE
    if _NC_CACHE is None:
        _NC_CACHE = build_bass()
    return _NC_CACHE


def make_in_maps(x, W, b, centroids):
    x = np.asarray(x, dtype=np.float32)
    W = np.asarray(W, dtype=np.float32)
    b = np.asarray(b, dtype=np.float32)
    centroids = np.asarray(centroids, dtype=np.float32)

    wt_host = np.ascontiguousarray(W.T)  # [S, H]
    brow = (np.float32(B) * b).reshape(1, H).astype(np.float32)
    cent_host = np.ascontiguousarray(centroids)

    # Two-step host transpose [B,S,V] -> [S,V,B]: one pass to [S,B,V]
    # (contiguous 2KB runs, fast), then per-s [B,VL] -> [VL,B] blocks that
    # stay cache-resident. Direct one-shot transpose would thrash DRAM.
    xsb = np.ascontiguousarray(x.transpose(1, 0, 2))  # [S, B, V]
    in_maps = []
    for i in range(NCORES):
        xs_i = np.ascontiguousarray(
            xsb[:, :, i * VL : (i + 1) * VL].transpose(0, 2, 1)
        )  # [S, VL, B]
        in_maps.append({"xs": xs_i, "wt": wt_host, "bb": brow, "cent": cent_host})
    return in_maps


def run(inputs: dict, trace: bool = False):
    """Run on the 8 NeuronCores; returns (full_output, BassKernelResults)."""
    nc = _get_nc()
    in_maps = make_in_maps(**inputs)
    res = run_bass_kernel_spmd(nc, in_maps, list(range(NCORES)), trace=trace)
    full = np.concatenate([r["out"] for r in res.results], axis=0)
    return full, res


def kernel(x, W, b, centroids) -> np.ndarray:
    full, _ = run({"x": x, "W": W, "b": b, "centroids": centroids})
    return full


# revision 15
# speedup vs baseline: 1.2343x; 1.0903x over previous
"""HardClusterAssigner Trainium2 kernel.

Reference computation:
    x_emb = mean_b(einsum('bsv,hs->bvh', x, W) + b)   # [V, H]
    assignments = one_hot(argmin(-l2norm(x_emb) @ l2norm(centroids).T))

Key transformations used here:
  1. mean over B commutes with the (linear) contraction over S:
         mean_b(x @ W.T) = (mean_b x) @ W.T
     so the 34-GFLOP batched matmul collapses to a memory-bound reduction
     of x over B followed by one [V,S]x[S,H] matmul.
  2. l2norm of the embedding is a positive per-row scale -> it cannot change
     the row-wise argmin, so it is skipped. Only centroids need normalizing.
  3. The 1/B mean scale and the bias are folded in exactly:
         B * (mean_b(xW.T) + bias) = (sum_b x) @ W.T + B*bias
     and the overall positive factor B is again argmin-invariant.

Sharding: V (last dim of x) is split across the 8 cores; every stage after
the split is core-local (no collectives). Each core computes its 64 rows of
the one-hot output.
"""

import sys

for _p in ("/opt/trn_rl_repo",):
    if _p not in sys.path:
        sys.path.append(_p)

from contextlib import ExitStack

import numpy as np

import concourse.bacc as bacc
import concourse.bass as bass
import concourse.mybir as mybir
from concourse import tile
from concourse.bass_utils import run_bass_kernel_spmd
from concourse.masks import make_identity

B, S, V, H, C = 64, 1024, 512, 512, 64
NCORES = 8
VL = V // NCORES  # 64 V-columns per core
P = 128
ST = S // P  # 8 s-chunks
F32 = mybir.dt.float32

_NC_CACHE = None


def build_bass() -> bass.Bass:
    nc = bacc.Bacc("TRN2", target_bir_lowering=False)

    xs = nc.declare_dram_parameter("xs", [S, VL, B], F32, isOutput=False)
    wt = nc.declare_dram_parameter("wt", [S, H], F32, isOutput=False)
    bb = nc.declare_dram_parameter("bb", [1, H], F32, isOutput=False)
    cent = nc.declare_dram_parameter("cent", [C, H], F32, isOutput=False)
    out = nc.declare_dram_parameter("out", [VL, C], F32, isOutput=True)

    with tile.TileContext(nc) as tc, ExitStack() as ctx:
        consts = ctx.enter_context(tc.tile_pool(name="consts", bufs=1))
        xpool = ctx.enter_context(tc.tile_pool(name="x", bufs=8))
        xmpool = ctx.enter_context(tc.tile_pool(name="xm", bufs=1))
        spool = ctx.enter_context(tc.tile_pool(name="small", bufs=1))
        psum = ctx.enter_context(tc.tile_pool(name="psum", bufs=1, space="PSUM"))
        tpsum = ctx.enter_context(tc.tile_pool(name="tpsum", bufs=2, space="PSUM"))

        # --- constants / small inputs -------------------------------------
        # const DMAs ride the ACT HWDGE ring so x tiles own the SP ring;
        # centroids first (needed by the mid-stream normalize), W.T last.
        centt = spool.tile([C, H], F32)
        nc.scalar.dma_start(out=centt[:], in_=cent[:])
        brow = consts.tile([1, H], F32)
        nc.scalar.dma_start(out=brow[:], in_=bb[:])
        wtr = consts.tile([P, ST, H], F32)  # W.T tiled: [:, t, :] is s-chunk t
        nc.scalar.dma_start(out=wtr[:], in_=wt.rearrange("(t p) h -> p t h", p=P))

        ones_row = consts.tile([1, VL], F32)
        nc.vector.memset(ones_row[:], 1.0)

        ident = consts.tile([P, P], F32)
        make_identity(nc, ident[:])

        # centroid row norms: square+row-sum fused on ACT (cheap, early)
        csq = spool.tile([C, H], F32)
        ssq = spool.tile([C, 1], F32)
        nc.scalar.activation(
            csq[:], centt[:], mybir.ActivationFunctionType.Square, accum_out=ssq[:]
        )
        cnorm = spool.tile([C, 1], F32)
        nc.scalar.sqrt(cnorm[:], ssq[:])
        cinv = spool.tile([C, 1], F32)
        centn = spool.tile([C, H], F32)
        cenT = spool.tile([P, 4 * C], F32)

        # --- x: DMA + reduce over B, then accumulate matmuls --------------
        # xs[s, v, b]; tile t holds s in [t*128, (t+1)*128); b innermost so
        # the reduce streams unit-stride. Two v-halves per s-chunk (1MiB DMAs)
        # for finer DMA/DVE pipelining.
        HV = VL // 2  # 32
        xs_r = xs.rearrange("(t p) v b -> t p (v b)", p=P)
        emb_ps = psum.tile([VL, H], F32, tag="emb")
        # bias first (opens the accumulation group): ones.T @ (B*b)
        nc.tensor.matmul(emb_ps[:], ones_row[:], brow[:], start=True, stop=False)
        for t in range(ST):
            xm = xmpool.tile([P, VL], F32, tag=f"xm{t}")
            for h in range(2):
                xt = xpool.tile([P, HV * B], F32, tag="xt")
                nc.sync.dma_start(out=xt[:], in_=xs_r[t][:, h * HV * B : (h + 1) * HV * B])
                nc.vector.tensor_reduce(
                    xm[:, h * HV : (h + 1) * HV],
                    xt[:].rearrange("p (v b) -> p v b", b=B),
                    axis=mybir.AxisListType.X,
                    op=mybir.AluOpType.add,
                )
            nc.tensor.matmul(
                emb_ps[:],
                xm[:],
                wtr[:, t, :],
                start=False,
                stop=(t == ST - 1),
            )
            if t == 2:
                # Centroid normalize + transpose, tucked into the DVE/PE
                # slack while the x stream is DMA-bound. By now the
                # centroid DMA + ACT norm ops have long finished.
                nc.vector.reciprocal(cinv[:], cnorm[:])
                nc.vector.tensor_scalar_mul(centn[:], centt[:], cinv[:])
                for k in range(4):
                    cp = tpsum.tile([P, C], F32, tag="tp")
                    nc.tensor.transpose(
                        cp[:], centn[:, k * P : (k + 1) * P], ident[:C, :C]
                    )
                    nc.scalar.copy(cenT[:, k * C : (k + 1) * C], cp[:])

        emb_sb = spool.tile([VL, H], F32)
        nc.scalar.copy(emb_sb[:], emb_ps[:])

        # --- similarity = emb @ centn.T via PE (contract H on partitions) -
        embT = spool.tile([P, 4 * VL], F32)
        for k in range(4):
            ep = tpsum.tile([P, VL], F32, tag="tp")
            nc.tensor.transpose(ep[:], emb_sb[:, k * P : (k + 1) * P], ident[:VL, :VL])
            nc.scalar.copy(embT[:, k * VL : (k + 1) * VL], ep[:])

        sim_ps = psum.tile([VL, C], F32, tag="sim")
        for k in range(4):
            nc.tensor.matmul(
                sim_ps[:],
                embT[:, k * VL : (k + 1) * VL],
                cenT[:, k * C : (k + 1) * C],
                start=(k == 0),
                stop=(k == 3),
            )

        # --- one-hot of row argmax ----------------------------------------
        mx = spool.tile([VL, 1], F32)
        nc.vector.tensor_reduce(
            mx[:], sim_ps[:], axis=mybir.AxisListType.X, op=mybir.AluOpType.max
        )
        oh = spool.tile([VL, C], F32)
        nc.vector.tensor_scalar(
            oh[:], sim_ps[:], mx[:], None, op0=mybir.AluOpType.is_equal
        )
        nc.sync.dma_start(out=out[:], in_=oh[:])

    nc.compile()
    return nc


def _get_nc() -> bass.Bass:
    global _NC_CACHE
    if _NC_CACHE is None:
        _NC_CACHE = build_bass()
    return _NC_CACHE


def make_in_maps(x, W, b, centroids):
    x = np.asarray(x, dtype=np.float32)
    W = np.asarray(W, dtype=np.float32)
    b = np.asarray(b, dtype=np.float32)
    centroids = np.asarray(centroids, dtype=np.float32)

    wt_host = np.ascontiguousarray(W.T)  # [S, H]
    brow = (np.float32(B) * b).reshape(1, H).astype(np.float32)
    cent_host = np.ascontiguousarray(centroids)

    # Two-step host transpose [B,S,V] -> [S,V,B]: one pass to [S,B,V]
    # (contiguous 2KB runs, fast), then per-s [B,VL] -> [VL,B] blocks that
    # stay cache-resident. Direct one-shot transpose would thrash DRAM.
    xsb = np.ascontiguousarray(x.transpose(1, 0, 2))  # [S, B, V]
    in_maps = []
    for i in range(NCORES):
        xs_i = np.ascontiguousarray(
            xsb[:, :, i * VL : (i + 1) * VL].transpose(0, 2, 1)
        )  # [S, VL, B]
        in_maps.append({"xs": xs_i, "wt": wt_host, "bb": brow, "cent": cent_host})
    return in_maps


def run(inputs: dict, trace: bool = False):
    """Run on the 8 NeuronCores; returns (full_output, BassKernelResults)."""
    nc = _get_nc()
    in_maps = make_in_maps(**inputs)
    res = run_bass_kernel_spmd(nc, in_maps, list(range(NCORES)), trace=trace)
    full = np.concatenate([r["out"] for r in res.results], axis=0)
    return full, res


def kernel(x, W, b, centroids) -> np.ndarray:
    full, _ = run({"x": x, "W": W, "b": b, "centroids": centroids})
    return full
